# revision 33
# baseline (speedup 1.0000x reference)
"""Trainium2 Bass kernel for nn_BilinearInterpolator (dense per-coord CNN).

Math (per (b, n) pair):
  u      = w1[:, :5] @ [image_b; pos]              # [64, 1024], shared over n
  v      = w1[:, 5:] @ coords[b, n] + b1           # [64] per-pair bias
  h1     = leaky(u + v)                            # [64, 1024]
  h_l    = leaky(W_l h_{l-1} + b_l)   l = 2..5
  pooled = mean_hw(h5);  out = sigmoid(wl @ pooled + bl)

Sharding: 512 (b, n) pairs data-parallel over 8 cores (64 pairs each; every
core owns a single b). On-chip layout packs 2 pairs per 128-partition tile
(channels 0-63 = even pair, 64-127 = odd pair); all matmuls use block-diagonal
[128, 128] weights. LeakyReLU+bias is fused into one ScalarE Prelu op per
tile; a tuned subset of tiles runs on VectorE instead (fp16 add-bias, then a
4x-mode is_ge/max mask and a 2x-mode multiply) to split the elementwise
bottleneck across both engines. Stages are emitted in a skewed wavefront so
each engine FIFO interleaves independent packs (no same-chain stalls, dense
PE bursts). The layer-5 activation's accum_out provides the spatial sum
(pooling) for free; the sigmoid head is one tiny matmul + activation.
"""

import sys

if "/opt/trn_rl_repo" not in sys.path:
    sys.path.insert(0, "/opt/trn_rl_repo")

import ml_dtypes
import numpy as np

import concourse.mybir as mybir
from concourse.bacc import Bacc
from concourse import tile
from concourse.bass_utils import run_bass_kernel_spmd

B, N, H, W, C = 4, 128, 32, 32, 64
HW = H * W
NCORES = 8
PAIRS = (B * N) // NCORES  # 64 pairs per core
PACKS = PAIRS // 2  # 32 packed tiles per core
NEG = 0.1
F32 = mybir.dt.float32
BF16 = mybir.dt.bfloat16
MM_MODE = "f16"  # "bf16" | "f16" | "f32r" | "f32"
MM_DT = {"bf16": BF16, "f16": mybir.dt.float16, "f32r": mybir.dt.float32r, "f32": F32}[MM_MODE]


def _r(ap):
    return ap

A = mybir.ActivationFunctionType
OP = mybir.AluOpType

# Engine ownership of the per-tile activation. Layer 5 is always ScalarE
# (needs accum_out). Tuned from trace balancing: ScalarE is ~2.4x more
# efficient per tile, but it also owns all of layer 5.
SKEW = 3  # pack skew between consecutive layers in emission order


ACT_L1 = 6


def _l1_owner(t):
    return "act" if t < ACT_L1 else "dve"


def _dve_owned(layer, t):
    if layer == 1:
        return _l1_owner(t) == "dve"
    if layer == 5:
        return t >= 27
    return (t + layer) % 3 == 0


def _build():
    nc = Bacc()
    d = {}
    for name, shape in [
        ("xin", [5, HW]),
        ("crd", [4, PACKS]),
        ("wu", [5, 128]),
        ("wc", [4, 128]),
                ("bball", [128, 4]),
        ("bb1", [128, 1]),
        ("wh", [128, 6]),
        ("bbl", [6, 1]),
    ]:
        d[name] = nc.dram_tensor(name, shape, F32, kind="ExternalInput")
    d["wall"] = nc.dram_tensor("wall", [128, 4 * 128], MM_DT, kind="ExternalInput")
    out_d = nc.dram_tensor("out", [6, PACKS], F32, kind="ExternalOutput")

    with tile.TileContext(nc) as tc:
        with (
            tc.tile_pool(name="consts", bufs=1) as consts,
            tc.tile_pool(name="hpool", bufs=34) as hpool,
            tc.tile_pool(name="apool", bufs=6) as apool,
            tc.tile_pool(name="mpool", bufs=6) as mpool,
            tc.tile_pool(name="s5pool", bufs=2) as s5pool,
            tc.tile_pool(name="zpool", bufs=4, space="PSUM") as zpool,
        ):
            sb = {}
            for name in d:
                if name == "out":
                    continue
                sb[name] = consts.tile(list(d[name].shape), d[name].dtype, tag=name, name="sb_" + name)
                nc.sync.dma_start(sb[name][:], d[name][:])

            w_l = {l: sb["wall"][:, 128 * (l - 2) : 128 * (l - 1)] for l in (2, 3, 4, 5)}
            bb_l = {l: sb["bball"][:, (l - 2) : (l - 1)] for l in (2, 3, 4, 5)}

            # per-pair input bias first (layer-1 ops need it earliest)
            zpc = zpool.tile([128, PACKS], F32, tag="z")
            nc.tensor.matmul(zpc[:], sb["wc"][:], sb["crd"][:])
            bias1 = consts.tile([128, PACKS], F32, tag="bias1")
            nc.scalar.activation(bias1[:], zpc[:], A.Identity, bias=sb["bb1"][:])

            # u = first conv applied to [image; pos], duplicated to both
            # partition halves by the doubled-column lhsT. Copy halves on
            # both elementwise engines so the ramp is parallel.
            zpu = zpool.tile([128, HW], F32, tag="z")
            nc.tensor.matmul(zpu[:, 0:512], sb["wu"][:], sb["xin"][:, 0:512])
            nc.tensor.matmul(zpu[:, 512:1024], sb["wu"][:], sb["xin"][:, 512:1024])
            u_dup = consts.tile([128, HW], F32, tag="u_dup")
            nc.scalar.copy(u_dup[:, 0:512], zpu[:, 0:512])
            nc.vector.tensor_scalar(
                u_dup[:, 512:1024], zpu[:, 512:1024], 1.0, None, OP.mult
            )

            pooled = consts.tile([128, PACKS], F32, tag="pooled")

            # Wavefront emission: stage (l, t) is emitted at wave
            # t + SKEW*(l-1), so every engine FIFO interleaves packs and
            # layers; consecutive ops on one engine never belong to the same
            # dependency chain, and all five layers stay in flight at once.
            hcur = {}

            def stage(l, t):
                if l == 1:
                    own = _l1_owner(t)
                    if own == "act":
                        h = hpool.tile([128, HW], MM_DT, tag="h", name=f"h1_{t}")
                        nc.scalar.activation(
                            h[:], u_dup[:], A.Prelu,
                            bias=bias1[:, t : t + 1], scale=1.0, alpha=NEG,
                        )
                    else:
                        a = apool.tile([128, HW], MM_DT, tag="a", name=f"a_{t}")
                        nc.vector.tensor_scalar(
                            a[:], u_dup[:], bias1[:, t : t + 1], None, OP.add
                        )
                        m = mpool.tile([128, HW], MM_DT, tag="m", name=f"m1_{t}")
                        nc.vector.tensor_scalar(
                            m[:], a[:], 0.0, NEG, OP.is_ge, OP.max
                        )
                        h = hpool.tile([128, HW], MM_DT, tag="h", name=f"h1_{t}")
                        nc.vector.tensor_tensor(h[:], a[:], m[:], OP.mult)
                    hcur[t] = h
                    return
                # layers 2..5: quadrant-packed matmuls over a pack pair.
                # Roles: tp streams rhs[0:64]->out[0:64] / rhs[64:]->out[64:],
                # tq streams rhs[0:64]->out[64:] / rhs[64:]->out[0:64] (pair
                # order inside tq's tiles swaps every layer; it swaps back by
                # layer 5 and biases are half-symmetric, so it is transparent).
                tp, tq = t, t + 1
                zs = {}
                for tt in (tp, tq):
                    h = hcur[tt]
                    z = zpool.tile([128, HW], F32, tag="z", name=f"z{l}_{tt}")
                    for c0 in (0, 512):
                        nc.tensor.matmul(
                            z[:, c0 : c0 + 512], _r(w_l[l]), _r(h[:, c0 : c0 + 512]),
                            start=True, stop=True, skip_group_check=True,
                        )
                    zs[tt] = z
                for tt in (tp, tq):
                    z = zs[tt]
                    if _dve_owned(l, tt):
                        a = apool.tile([128, HW], MM_DT, tag="a", name=f"a{l}_{tt}")
                        nc.vector.tensor_scalar(
                            a[:], z[:], bb_l[l], None, OP.add
                        )
                        m = mpool.tile([128, HW], MM_DT, tag="m", name=f"m{l}_{tt}")
                        nc.vector.tensor_scalar(
                            m[:], a[:], 0.0, NEG, OP.is_ge, OP.max
                        )
                        hn = hpool.tile([128, HW], MM_DT, tag="h", name=f"h{l}_{tt}")
                        if l == 5:
                            nc.vector.scalar_tensor_tensor(
                                hn[:], a[:], 1.0, m[:], OP.mult, OP.mult,
                                accum_out=pooled[:, tt : tt + 1],
                            )
                        else:
                            nc.vector.tensor_tensor(hn[:], a[:], m[:], OP.mult)
                    else:
                        if l == 5:
                            hn = s5pool.tile([128, HW], MM_DT, tag="s5", name=f"h5_{tt}")
                            nc.scalar.activation(
                                hn[:], z[:], A.Prelu,
                                bias=bb_l[l], scale=1.0, alpha=NEG,
                                accum_out=pooled[:, tt : tt + 1],
                            )
                        else:
                            hn = hpool.tile([128, HW], MM_DT, tag="h", name=f"h{l}_{tt}")
                            nc.scalar.activation(
                                hn[:], z[:], A.Prelu,
                                bias=bb_l[l], scale=1.0, alpha=NEG,
                            )
                    hcur[tt] = hn

            for w in range(PACKS + SKEW * 4):
                for l in (1, 2, 3, 4, 5):
                    t = w - SKEW * (l - 1)
                    if 0 <= t < PACKS and (l == 1 or t % 2 == 0):
                        stage(l, t)

            # ---- head ----
            zph = zpool.tile([6, PACKS], F32, tag="z")
            nc.tensor.matmul(zph[:], sb["wh"][:], pooled[:])
            out_sb = consts.tile([6, PACKS], F32, tag="out_sb")
            nc.scalar.activation(out_sb[:], zph[:], A.Sigmoid, bias=sb["bbl"][:])
            nc.sync.dma_start(out_d[:], out_sb[:])

    nc.compile()
    return nc


_CACHE = {}


def _get_nc():
    if "nc" not in _CACHE:
        _CACHE["nc"] = _build()
    return _CACHE["nc"]


def _prep_core_inputs(image, coords, w1, b1, ws, bs, wl, bl, core):
    b = core // 2
    n0 = (core % 2) * PAIRS

    row = (np.arange(H, dtype=np.float32) / (H - 1))[:, None] * np.ones(
        (1, W), np.float32
    )
    col = np.ones((H, 1), np.float32) * (np.arange(W, dtype=np.float32) / (W - 1))[None]
    pos = np.stack([row, col], 0).reshape(2, HW)
    xin = np.concatenate([image[b].reshape(3, HW), pos], 0)

    cs = coords[b, n0 : n0 + PAIRS]  # [64, 2]
    crd = np.stack([cs[0::2, 0], cs[0::2, 1], cs[1::2, 0], cs[1::2, 1]], 0)

    w1aT = np.ascontiguousarray(w1[:, :5].T)  # [5, 64]
    w1bT = np.ascontiguousarray(w1[:, 5:].T)  # [2, 64]
    wu = np.concatenate([w1aT, w1aT], 1)  # [5, 128]
    wc = np.zeros((4, 128), np.float32)
    wc[0:2, 0:64] = w1bT
    wc[2:4, 64:128] = w1bT

    wall = np.zeros((128, 4 * 128), np.float32)
    bball = np.zeros((128, 4), np.float32)
    for i, (w, bias) in enumerate(zip(ws, bs)):
        wall[0:64, 128 * i : 128 * i + 64] = w.T
        wall[64:128, 128 * i + 64 : 128 * i + 128] = w.T
        bball[:, i] = np.concatenate([bias, bias])

    wh = np.zeros((128, 6), np.float32)
    wh[0:64, 0:3] = wl.T / HW
    wh[64:128, 3:6] = wl.T / HW

    return {
        "xin": np.ascontiguousarray(xin, np.float32),
        "crd": np.ascontiguousarray(crd, np.float32),
        "wu": np.ascontiguousarray(wu, np.float32),
        "wc": wc,
        "wall": {"bf16": wall.astype(ml_dtypes.bfloat16), "f16": wall.astype(np.float16)}.get(MM_MODE, wall),
        "bball": bball,
        "bb1": np.concatenate([b1, b1]).reshape(128, 1).astype(np.float32),
        "wh": wh,
        "bbl": np.concatenate([bl, bl]).reshape(6, 1).astype(np.float32),
    }


def _run(inputs, trace=False):
    image = np.asarray(inputs["image"], np.float32)
    coords = np.asarray(inputs["coords"], np.float32)
    w1 = np.asarray(inputs["w1"], np.float32)
    b1 = np.asarray(inputs["b1"], np.float32)
    ws = [np.asarray(inputs[f"w{i}"], np.float32) for i in (2, 3, 4, 5)]
    bs = [np.asarray(inputs[f"b{i}"], np.float32) for i in (2, 3, 4, 5)]
    wl = np.asarray(inputs["wl"], np.float32)
    bl = np.asarray(inputs["bl"], np.float32)

    nc = _get_nc()
    in_maps = [
        _prep_core_inputs(image, coords, w1, b1, ws, bs, wl, bl, c)
        for c in range(NCORES)
    ]
    res = run_bass_kernel_spmd(nc, in_maps, list(range(NCORES)), trace=trace)

    pred = np.empty((B, 3, N), np.float32)
    for c in range(NCORES):
        b = c // 2
        n0 = (c % 2) * PAIRS
        o = res.results[c]["out"]  # [6, 32]
        pred[b, :, n0 + 0 : n0 + PAIRS : 2] = o[0:3]
        pred[b, :, n0 + 1 : n0 + PAIRS : 2] = o[3:6]
    return pred, res


def kernel(**inputs) -> np.ndarray:
    pred, _ = _run(inputs, trace=False)
    return pred


# revision 34
# speedup vs baseline: 1.0234x; 1.0234x over previous
"""Trainium2 Bass kernel for nn_BilinearInterpolator (dense per-coord CNN).

Math (per (b, n) pair):
  u      = w1[:, :5] @ [image_b; pos]              # [64, 1024], shared over n
  v      = w1[:, 5:] @ coords[b, n] + b1           # [64] per-pair bias
  h1     = leaky(u + v)                            # [64, 1024]
  h_l    = leaky(W_l h_{l-1} + b_l)   l = 2..5
  pooled = mean_hw(h5);  out = sigmoid(wl @ pooled + bl)

Sharding: 512 (b, n) pairs data-parallel over 8 cores (64 pairs each; every
core owns a single b). On-chip layout packs 2 pairs per 128-partition tile
(channels 0-63 = even pair, 64-127 = odd pair); all matmuls use block-diagonal
[128, 128] weights. LeakyReLU+bias is fused into one ScalarE Prelu op per
tile; a tuned subset of tiles runs on VectorE instead (fp16 add-bias, then a
4x-mode is_ge/max mask and a 2x-mode multiply) to split the elementwise
bottleneck across both engines. Stages are emitted in a skewed wavefront so
each engine FIFO interleaves independent packs (no same-chain stalls, dense
PE bursts). The layer-5 activation's accum_out provides the spatial sum
(pooling) for free; the sigmoid head is one tiny matmul + activation.
"""

import sys

if "/opt/trn_rl_repo" not in sys.path:
    sys.path.insert(0, "/opt/trn_rl_repo")

import ml_dtypes
import numpy as np

import concourse.mybir as mybir
from concourse.bacc import Bacc
from concourse import tile
from concourse.bass_utils import run_bass_kernel_spmd

B, N, H, W, C = 4, 128, 32, 32, 64
HW = H * W
NCORES = 8
PAIRS = (B * N) // NCORES  # 64 pairs per core
PACKS = PAIRS // 2  # 32 packed tiles per core
NEG = 0.1
F32 = mybir.dt.float32
BF16 = mybir.dt.bfloat16
MM_MODE = "f16"  # "bf16" | "f16" | "f32r" | "f32"
MM_DT = {"bf16": BF16, "f16": mybir.dt.float16, "f32r": mybir.dt.float32r, "f32": F32}[MM_MODE]


def _r(ap):
    return ap

A = mybir.ActivationFunctionType
OP = mybir.AluOpType

# Engine ownership of the per-tile activation. Layer 5 is always ScalarE
# (needs accum_out). Tuned from trace balancing: ScalarE is ~2.4x more
# efficient per tile, but it also owns all of layer 5.
SKEW = 3  # pack skew between consecutive layers in emission order


ACT_L1 = 6


def _l1_owner(t):
    return "act" if t < ACT_L1 else "dve"


def _dve_owned(layer, t):
    if layer == 1:
        return _l1_owner(t) == "dve"
    if layer == 5:
        return False
    return (t + layer) % 3 == 0


def _build():
    nc = Bacc()
    d = {}
    for name, shape in [
        ("xin", [5, HW]),
        ("crd", [4, PACKS]),
        ("wu", [5, 128]),
        ("wc", [4, 128]),
                ("bball", [128, 4]),
        ("bb1", [128, 1]),
        ("wh", [128, 6]),
        ("bbl", [6, 1]),
    ]:
        d[name] = nc.dram_tensor(name, shape, F32, kind="ExternalInput")
    d["wall"] = nc.dram_tensor("wall", [128, 4 * 128], MM_DT, kind="ExternalInput")
    out_d = nc.dram_tensor("out", [6, PACKS], F32, kind="ExternalOutput")

    with tile.TileContext(nc) as tc:
        with (
            tc.tile_pool(name="consts", bufs=1) as consts,
            tc.tile_pool(name="hpool", bufs=34) as hpool,
            tc.tile_pool(name="apool", bufs=6) as apool,
            tc.tile_pool(name="mpool", bufs=6) as mpool,
            tc.tile_pool(name="s5pool", bufs=2) as s5pool,
            tc.tile_pool(name="zpool", bufs=4, space="PSUM") as zpool,
        ):
            sb = {}
            for name in d:
                if name == "out":
                    continue
                sb[name] = consts.tile(list(d[name].shape), d[name].dtype, tag=name, name="sb_" + name)
                nc.sync.dma_start(sb[name][:], d[name][:])

            w_l = {l: sb["wall"][:, 128 * (l - 2) : 128 * (l - 1)] for l in (2, 3, 4, 5)}
            bb_l = {l: sb["bball"][:, (l - 2) : (l - 1)] for l in (2, 3, 4, 5)}

            # per-pair input bias first (layer-1 ops need it earliest)
            zpc = zpool.tile([128, PACKS], F32, tag="z")
            nc.tensor.matmul(zpc[:], sb["wc"][:], sb["crd"][:])
            bias1 = consts.tile([128, PACKS], F32, tag="bias1")
            nc.scalar.activation(bias1[:], zpc[:], A.Identity, bias=sb["bb1"][:])

            # u = first conv applied to [image; pos], duplicated to both
            # partition halves by the doubled-column lhsT. Copy halves on
            # both elementwise engines so the ramp is parallel.
            zpu = zpool.tile([128, HW], F32, tag="z")
            nc.tensor.matmul(zpu[:, 0:512], sb["wu"][:], sb["xin"][:, 0:512])
            nc.tensor.matmul(zpu[:, 512:1024], sb["wu"][:], sb["xin"][:, 512:1024])
            u_dup = consts.tile([128, HW], F32, tag="u_dup")
            nc.scalar.copy(u_dup[:, 0:512], zpu[:, 0:512])
            nc.vector.tensor_scalar(
                u_dup[:, 512:1024], zpu[:, 512:1024], 1.0, None, OP.mult
            )

            pooled = consts.tile([128, PACKS], F32, tag="pooled")

            # Wavefront emission: stage (l, t) is emitted at wave
            # t + SKEW*(l-1), so every engine FIFO interleaves packs and
            # layers; consecutive ops on one engine never belong to the same
            # dependency chain, and all five layers stay in flight at once.
            hcur = {}

            def stage(l, t):
                if l == 1:
                    own = _l1_owner(t)
                    if own == "act":
                        h = hpool.tile([128, HW], MM_DT, tag="h", name=f"h1_{t}")
                        nc.scalar.activation(
                            h[:], u_dup[:], A.Prelu,
                            bias=bias1[:, t : t + 1], scale=1.0, alpha=NEG,
                        )
                    else:
                        a = apool.tile([128, HW], MM_DT, tag="a", name=f"a_{t}")
                        nc.vector.tensor_scalar(
                            a[:], u_dup[:], bias1[:, t : t + 1], None, OP.add
                        )
                        m = mpool.tile([128, HW], MM_DT, tag="m", name=f"m1_{t}")
                        nc.vector.tensor_scalar(
                            m[:], a[:], 0.0, NEG, OP.is_ge, OP.max
                        )
                        h = hpool.tile([128, HW], MM_DT, tag="h", name=f"h1_{t}")
                        nc.vector.tensor_tensor(h[:], a[:], m[:], OP.mult)
                    hcur[t] = h
                    return
                # layers 2..5: quadrant-packed matmuls over a pack pair.
                # Roles: tp streams rhs[0:64]->out[0:64] / rhs[64:]->out[64:],
                # tq streams rhs[0:64]->out[64:] / rhs[64:]->out[0:64] (pair
                # order inside tq's tiles swaps every layer; it swaps back by
                # layer 5 and biases are half-symmetric, so it is transparent).
                tp, tq = t, t + 1
                zs = {}
                for tt in (tp, tq):
                    h = hcur[tt]
                    z = zpool.tile([128, HW], F32, tag="z", name=f"z{l}_{tt}")
                    for c0 in (0, 512):
                        nc.tensor.matmul(
                            z[:, c0 : c0 + 512], _r(w_l[l]), _r(h[:, c0 : c0 + 512]),
                            start=True, stop=True, skip_group_check=True,
                        )
                    zs[tt] = z
                for tt in (tp, tq):
                    z = zs[tt]
                    if _dve_owned(l, tt):
                        a = apool.tile([128, HW], MM_DT, tag="a", name=f"a{l}_{tt}")
                        nc.vector.tensor_scalar(
                            a[:], z[:], bb_l[l], None, OP.add
                        )
                        m = mpool.tile([128, HW], MM_DT, tag="m", name=f"m{l}_{tt}")
                        nc.vector.tensor_scalar(
                            m[:], a[:], 0.0, NEG, OP.is_ge, OP.max
                        )
                        hn = hpool.tile([128, HW], MM_DT, tag="h", name=f"h{l}_{tt}")
                        if l == 5:
                            nc.vector.scalar_tensor_tensor(
                                hn[:], a[:], 1.0, m[:], OP.mult, OP.mult,
                                accum_out=pooled[:, tt : tt + 1],
                            )
                        else:
                            nc.vector.tensor_tensor(hn[:], a[:], m[:], OP.mult)
                    else:
                        if l == 5:
                            hn = s5pool.tile([128, HW], MM_DT, tag="s5", name=f"h5_{tt}")
                            nc.scalar.activation(
                                hn[:], z[:], A.Prelu,
                                bias=bb_l[l], scale=1.0, alpha=NEG,
                                accum_out=pooled[:, tt : tt + 1],
                            )
                        else:
                            hn = hpool.tile([128, HW], MM_DT, tag="h", name=f"h{l}_{tt}")
                            nc.scalar.activation(
                                hn[:], z[:], A.Prelu,
                                bias=bb_l[l], scale=1.0, alpha=NEG,
                            )
                    hcur[tt] = hn

            for w in range(PACKS + SKEW * 4):
                for l in (1, 2, 3, 4, 5):
                    t = w - SKEW * (l - 1)
                    if 0 <= t < PACKS and (l == 1 or t % 2 == 0):
                        stage(l, t)

            # ---- head ----
            zph = zpool.tile([6, PACKS], F32, tag="z")
            nc.tensor.matmul(zph[:], sb["wh"][:], pooled[:])
            out_sb = consts.tile([6, PACKS], F32, tag="out_sb")
            nc.scalar.activation(out_sb[:], zph[:], A.Sigmoid, bias=sb["bbl"][:])
            nc.sync.dma_start(out_d[:], out_sb[:])

    nc.compile()
    return nc


_CACHE = {}


def _get_nc():
    if "nc" not in _CACHE:
        _CACHE["nc"] = _build()
    return _CACHE["nc"]


def _prep_core_inputs(image, coords, w1, b1, ws, bs, wl, bl, core):
    b = core // 2
    n0 = (core % 2) * PAIRS

    row = (np.arange(H, dtype=np.float32) / (H - 1))[:, None] * np.ones(
        (1, W), np.float32
    )
    col = np.ones((H, 1), np.float32) * (np.arange(W, dtype=np.float32) / (W - 1))[None]
    pos = np.stack([row, col], 0).reshape(2, HW)
    xin = np.concatenate([image[b].reshape(3, HW), pos], 0)

    cs = coords[b, n0 : n0 + PAIRS]  # [64, 2]
    crd = np.stack([cs[0::2, 0], cs[0::2, 1], cs[1::2, 0], cs[1::2, 1]], 0)

    w1aT = np.ascontiguousarray(w1[:, :5].T)  # [5, 64]
    w1bT = np.ascontiguousarray(w1[:, 5:].T)  # [2, 64]
    wu = np.concatenate([w1aT, w1aT], 1)  # [5, 128]
    wc = np.zeros((4, 128), np.float32)
    wc[0:2, 0:64] = w1bT
    wc[2:4, 64:128] = w1bT

    wall = np.zeros((128, 4 * 128), np.float32)
    bball = np.zeros((128, 4), np.float32)
    for i, (w, bias) in enumerate(zip(ws, bs)):
        wall[0:64, 128 * i : 128 * i + 64] = w.T
        wall[64:128, 128 * i + 64 : 128 * i + 128] = w.T
        bball[:, i] = np.concatenate([bias, bias])

    wh = np.zeros((128, 6), np.float32)
    wh[0:64, 0:3] = wl.T / HW
    wh[64:128, 3:6] = wl.T / HW

    return {
        "xin": np.ascontiguousarray(xin, np.float32),
        "crd": np.ascontiguousarray(crd, np.float32),
        "wu": np.ascontiguousarray(wu, np.float32),
        "wc": wc,
        "wall": {"bf16": wall.astype(ml_dtypes.bfloat16), "f16": wall.astype(np.float16)}.get(MM_MODE, wall),
        "bball": bball,
        "bb1": np.concatenate([b1, b1]).reshape(128, 1).astype(np.float32),
        "wh": wh,
        "bbl": np.concatenate([bl, bl]).reshape(6, 1).astype(np.float32),
    }


def _run(inputs, trace=False):
    image = np.asarray(inputs["image"], np.float32)
    coords = np.asarray(inputs["coords"], np.float32)
    w1 = np.asarray(inputs["w1"], np.float32)
    b1 = np.asarray(inputs["b1"], np.float32)
    ws = [np.asarray(inputs[f"w{i}"], np.float32) for i in (2, 3, 4, 5)]
    bs = [np.asarray(inputs[f"b{i}"], np.float32) for i in (2, 3, 4, 5)]
    wl = np.asarray(inputs["wl"], np.float32)
    bl = np.asarray(inputs["bl"], np.float32)

    nc = _get_nc()
    in_maps = [
        _prep_core_inputs(image, coords, w1, b1, ws, bs, wl, bl, c)
        for c in range(NCORES)
    ]
    res = run_bass_kernel_spmd(nc, in_maps, list(range(NCORES)), trace=trace)

    pred = np.empty((B, 3, N), np.float32)
    for c in range(NCORES):
        b = c // 2
        n0 = (c % 2) * PAIRS
        o = res.results[c]["out"]  # [6, 32]
        pred[b, :, n0 + 0 : n0 + PAIRS : 2] = o[0:3]
        pred[b, :, n0 + 1 : n0 + PAIRS : 2] = o[3:6]
    return pred, res


def kernel(**inputs) -> np.ndarray:
    pred, _ = _run(inputs, trace=False)
    return pred


# revision 37
# speedup vs baseline: 1.0414x; 1.0176x over previous
"""Trainium2 Bass kernel for nn_BilinearInterpolator (dense per-coord CNN).

Math (per (b, n) pair):
  u      = w1[:, :5] @ [image_b; pos]              # [64, 1024], shared over n
  v      = w1[:, 5:] @ coords[b, n] + b1           # [64] per-pair bias
  h1     = leaky(u + v)                            # [64, 1024]
  h_l    = leaky(W_l h_{l-1} + b_l)   l = 2..5
  pooled = mean_hw(h5);  out = sigmoid(wl @ pooled + bl)

Sharding: 512 (b, n) pairs data-parallel over 8 cores (64 pairs each; every
core owns a single b). On-chip layout packs 2 pairs per 128-partition tile
(channels 0-63 = even pair, 64-127 = odd pair); all matmuls use block-diagonal
[128, 128] weights. LeakyReLU+bias is fused into one ScalarE Prelu op per
tile; a tuned subset of tiles runs on VectorE instead (fp16 add-bias, then a
4x-mode is_ge/max mask and a 2x-mode multiply) to split the elementwise
bottleneck across both engines. Stages are emitted in a skewed wavefront so
each engine FIFO interleaves independent packs (no same-chain stalls, dense
PE bursts). The layer-5 activation's accum_out provides the spatial sum
(pooling) for free; the sigmoid head is one tiny matmul + activation.
"""

import sys

if "/opt/trn_rl_repo" not in sys.path:
    sys.path.insert(0, "/opt/trn_rl_repo")

import ml_dtypes
import numpy as np

import concourse.mybir as mybir
from concourse.bacc import Bacc
from concourse import tile
from concourse.bass_utils import run_bass_kernel_spmd

B, N, H, W, C = 4, 128, 32, 32, 64
HW = H * W
NCORES = 8
PAIRS = (B * N) // NCORES  # 64 pairs per core
PACKS = PAIRS // 2  # 32 packed tiles per core
NEG = 0.1
F32 = mybir.dt.float32
BF16 = mybir.dt.bfloat16
MM_MODE = "f16"  # "bf16" | "f16" | "f32r" | "f32"
MM_DT = {"bf16": BF16, "f16": mybir.dt.float16, "f32r": mybir.dt.float32r, "f32": F32}[MM_MODE]


def _r(ap):
    return ap

A = mybir.ActivationFunctionType
OP = mybir.AluOpType

# Engine ownership of the per-tile activation. Layer 5 is always ScalarE
# (needs accum_out). Tuned from trace balancing: ScalarE is ~2.4x more
# efficient per tile, but it also owns all of layer 5.
SKEW = 3  # pack skew between consecutive layers in emission order


ACT_L1 = 12


def _l1_owner(t):
    return "act" if t < ACT_L1 else "dve"


def _dve_owned(layer, t):
    if layer == 1:
        return _l1_owner(t) == "dve"
    if layer == 5:
        return False
    return (t + layer) % 3 == 0


def _build():
    nc = Bacc()
    d = {}
    for name, shape in [
        ("xin", [5, HW]),
        ("crd", [4, PACKS]),
        ("wu", [5, 128]),
        ("wc", [4, 128]),
                ("bball", [128, 4]),
        ("bb1", [128, 1]),
        ("wh", [128, 6]),
        ("bbl", [6, 1]),
    ]:
        d[name] = nc.dram_tensor(name, shape, F32, kind="ExternalInput")
    d["wall"] = nc.dram_tensor("wall", [128, 4 * 128], MM_DT, kind="ExternalInput")
    out_d = nc.dram_tensor("out", [6, PACKS], F32, kind="ExternalOutput")

    with tile.TileContext(nc) as tc:
        with (
            tc.tile_pool(name="consts", bufs=1) as consts,
            tc.tile_pool(name="hpool", bufs=34) as hpool,
            tc.tile_pool(name="apool", bufs=6) as apool,
            tc.tile_pool(name="mpool", bufs=6) as mpool,
            tc.tile_pool(name="s5pool", bufs=2) as s5pool,
            tc.tile_pool(name="zpool", bufs=4, space="PSUM") as zpool,
        ):
            sb = {}
            for name in d:
                if name == "out":
                    continue
                sb[name] = consts.tile(list(d[name].shape), d[name].dtype, tag=name, name="sb_" + name)
                nc.sync.dma_start(sb[name][:], d[name][:])

            w_l = {l: sb["wall"][:, 128 * (l - 2) : 128 * (l - 1)] for l in (2, 3, 4, 5)}
            bb_l = {l: sb["bball"][:, (l - 2) : (l - 1)] for l in (2, 3, 4, 5)}

            # per-pair input bias first (layer-1 ops need it earliest)
            zpc = zpool.tile([128, PACKS], F32, tag="z")
            nc.tensor.matmul(zpc[:], sb["wc"][:], sb["crd"][:])
            bias1 = consts.tile([128, PACKS], F32, tag="bias1")
            nc.scalar.activation(bias1[:], zpc[:], A.Identity, bias=sb["bb1"][:])

            # u = first conv applied to [image; pos], duplicated to both
            # partition halves by the doubled-column lhsT. Copy halves on
            # both elementwise engines so the ramp is parallel.
            zpu = zpool.tile([128, HW], F32, tag="z")
            nc.tensor.matmul(zpu[:, 0:512], sb["wu"][:], sb["xin"][:, 0:512])
            nc.tensor.matmul(zpu[:, 512:1024], sb["wu"][:], sb["xin"][:, 512:1024])
            u_dup = consts.tile([128, HW], F32, tag="u_dup")
            nc.scalar.copy(u_dup[:, 0:512], zpu[:, 0:512])
            nc.vector.tensor_scalar(
                u_dup[:, 512:1024], zpu[:, 512:1024], 1.0, None, OP.mult
            )

            pooled = consts.tile([128, PACKS], F32, tag="pooled")

            # Wavefront emission: stage (l, t) is emitted at wave
            # t + SKEW*(l-1), so every engine FIFO interleaves packs and
            # layers; consecutive ops on one engine never belong to the same
            # dependency chain, and all five layers stay in flight at once.
            hcur = {}

            def stage(l, t):
                if l == 1:
                    own = _l1_owner(t)
                    if own == "act":
                        h = hpool.tile([128, HW], MM_DT, tag="h", name=f"h1_{t}")
                        nc.scalar.activation(
                            h[:], u_dup[:], A.Prelu,
                            bias=bias1[:, t : t + 1], scale=1.0, alpha=NEG,
                        )
                    else:
                        a = apool.tile([128, HW], MM_DT, tag="a", name=f"a_{t}")
                        nc.vector.tensor_scalar(
                            a[:], u_dup[:], bias1[:, t : t + 1], None, OP.add
                        )
                        m = mpool.tile([128, HW], MM_DT, tag="m", name=f"m1_{t}")
                        nc.vector.tensor_scalar(
                            m[:], a[:], 0.0, NEG, OP.is_ge, OP.max
                        )
                        h = hpool.tile([128, HW], MM_DT, tag="h", name=f"h1_{t}")
                        nc.vector.tensor_tensor(h[:], a[:], m[:], OP.mult)
                    hcur[t] = h
                    return
                # layers 2..5: quadrant-packed matmuls over a pack pair.
                # Roles: tp streams rhs[0:64]->out[0:64] / rhs[64:]->out[64:],
                # tq streams rhs[0:64]->out[64:] / rhs[64:]->out[0:64] (pair
                # order inside tq's tiles swaps every layer; it swaps back by
                # layer 5 and biases are half-symmetric, so it is transparent).
                tp, tq = t, t + 1
                zs = {}
                for tt in (tp, tq):
                    h = hcur[tt]
                    z = zpool.tile([128, HW], F32, tag="z", name=f"z{l}_{tt}")
                    for c0 in (0, 512):
                        nc.tensor.matmul(
                            z[:, c0 : c0 + 512], _r(w_l[l]), _r(h[:, c0 : c0 + 512]),
                            start=True, stop=True, skip_group_check=True,
                        )
                    zs[tt] = z
                for tt in (tp, tq):
                    z = zs[tt]
                    if _dve_owned(l, tt):
                        a = apool.tile([128, HW], MM_DT, tag="a", name=f"a{l}_{tt}")
                        nc.vector.tensor_scalar(
                            a[:], z[:], bb_l[l], None, OP.add
                        )
                        m = mpool.tile([128, HW], MM_DT, tag="m", name=f"m{l}_{tt}")
                        nc.vector.tensor_scalar(
                            m[:], a[:], 0.0, NEG, OP.is_ge, OP.max
                        )
                        hn = hpool.tile([128, HW], MM_DT, tag="h", name=f"h{l}_{tt}")
                        if l == 5:
                            nc.vector.scalar_tensor_tensor(
                                hn[:], a[:], 1.0, m[:], OP.mult, OP.mult,
                                accum_out=pooled[:, tt : tt + 1],
                            )
                        else:
                            nc.vector.tensor_tensor(hn[:], a[:], m[:], OP.mult)
                    else:
                        if l == 5:
                            hn = s5pool.tile([128, HW], MM_DT, tag="s5", name=f"h5_{tt}")
                            nc.scalar.activation(
                                hn[:], z[:], A.Prelu,
                                bias=bb_l[l], scale=1.0, alpha=NEG,
                                accum_out=pooled[:, tt : tt + 1],
                            )
                        else:
                            hn = hpool.tile([128, HW], MM_DT, tag="h", name=f"h{l}_{tt}")
                            nc.scalar.activation(
                                hn[:], z[:], A.Prelu,
                                bias=bb_l[l], scale=1.0, alpha=NEG,
                            )
                    hcur[tt] = hn

            for w in range(PACKS + SKEW * 4):
                for l in (1, 2, 3, 4, 5):
                    t = w - SKEW * (l - 1)
                    if 0 <= t < PACKS and (l == 1 or t % 2 == 0):
                        stage(l, t)

            # ---- head ----
            zph = zpool.tile([6, PACKS], F32, tag="z")
            nc.tensor.matmul(zph[:], sb["wh"][:], pooled[:])
            out_sb = consts.tile([6, PACKS], F32, tag="out_sb")
            nc.scalar.activation(out_sb[:], zph[:], A.Sigmoid, bias=sb["bbl"][:])
            nc.sync.dma_start(out_d[:], out_sb[:])

    nc.compile()
    return nc


_CACHE = {}


def _get_nc():
    if "nc" not in _CACHE:
        _CACHE["nc"] = _build()
    return _CACHE["nc"]


def _prep_core_inputs(image, coords, w1, b1, ws, bs, wl, bl, core):
    b = core // 2
    n0 = (core % 2) * PAIRS

    row = (np.arange(H, dtype=np.float32) / (H - 1))[:, None] * np.ones(
        (1, W), np.float32
    )
    col = np.ones((H, 1), np.float32) * (np.arange(W, dtype=np.float32) / (W - 1))[None]
    pos = np.stack([row, col], 0).reshape(2, HW)
    xin = np.concatenate([image[b].reshape(3, HW), pos], 0)

    cs = coords[b, n0 : n0 + PAIRS]  # [64, 2]
    crd = np.stack([cs[0::2, 0], cs[0::2, 1], cs[1::2, 0], cs[1::2, 1]], 0)

    w1aT = np.ascontiguousarray(w1[:, :5].T)  # [5, 64]
    w1bT = np.ascontiguousarray(w1[:, 5:].T)  # [2, 64]
    wu = np.concatenate([w1aT, w1aT], 1)  # [5, 128]
    wc = np.zeros((4, 128), np.float32)
    wc[0:2, 0:64] = w1bT
    wc[2:4, 64:128] = w1bT

    wall = np.zeros((128, 4 * 128), np.float32)
    bball = np.zeros((128, 4), np.float32)
    for i, (w, bias) in enumerate(zip(ws, bs)):
        wall[0:64, 128 * i : 128 * i + 64] = w.T
        wall[64:128, 128 * i + 64 : 128 * i + 128] = w.T
        bball[:, i] = np.concatenate([bias, bias])

    wh = np.zeros((128, 6), np.float32)
    wh[0:64, 0:3] = wl.T / HW
    wh[64:128, 3:6] = wl.T / HW

    return {
        "xin": np.ascontiguousarray(xin, np.float32),
        "crd": np.ascontiguousarray(crd, np.float32),
        "wu": np.ascontiguousarray(wu, np.float32),
        "wc": wc,
        "wall": {"bf16": wall.astype(ml_dtypes.bfloat16), "f16": wall.astype(np.float16)}.get(MM_MODE, wall),
        "bball": bball,
        "bb1": np.concatenate([b1, b1]).reshape(128, 1).astype(np.float32),
        "wh": wh,
        "bbl": np.concatenate([bl, bl]).reshape(6, 1).astype(np.float32),
    }


def _run(inputs, trace=False):
    image = np.asarray(inputs["image"], np.float32)
    coords = np.asarray(inputs["coords"], np.float32)
    w1 = np.asarray(inputs["w1"], np.float32)
    b1 = np.asarray(inputs["b1"], np.float32)
    ws = [np.asarray(inputs[f"w{i}"], np.float32) for i in (2, 3, 4, 5)]
    bs = [np.asarray(inputs[f"b{i}"], np.float32) for i in (2, 3, 4, 5)]
    wl = np.asarray(inputs["wl"], np.float32)
    bl = np.asarray(inputs["bl"], np.float32)

    nc = _get_nc()
    in_maps = [
        _prep_core_inputs(image, coords, w1, b1, ws, bs, wl, bl, c)
        for c in range(NCORES)
    ]
    res = run_bass_kernel_spmd(nc, in_maps, list(range(NCORES)), trace=trace)

    pred = np.empty((B, 3, N), np.float32)
    for c in range(NCORES):
        b = c // 2
        n0 = (c % 2) * PAIRS
        o = res.results[c]["out"]  # [6, 32]
        pred[b, :, n0 + 0 : n0 + PAIRS : 2] = o[0:3]
        pred[b, :, n0 + 1 : n0 + PAIRS : 2] = o[3:6]
    return pred, res


def kernel(**inputs) -> np.ndarray:
    pred, _ = _run(inputs, trace=False)
    return pred
